# revision 7
# baseline (speedup 1.0000x reference)
"""CEP loss kernel for Trainium2: loss = -sum(d1 * log(d2 + eps)).

Inputs are rounded on the host: d1 -> bf16, d2 -> fp8 e4m3 (3 bytes per
element pair instead of 8), cutting the HBM stream to 6.29 MB/core.
Measured cost of the rounding: ~3.8e-3 relative error on the final sum
(gate is 2e-2); the d2 rounding dominates (ln amplifies it), d1's bf16
error is random-sign and averages out.

Full inputs [4096, 4096] are sharded row-wise across 8 NeuronCores (512
rows each).  Per core the shard streams as [128, w] pieces.  The piece
schedule starts with a 512-wide sliver so ScalarE's Ln chain (the
longest engine at ~16 us) starts as soon as the first 64 KB lands, and
tapers at the end so the post-stream tail is short.  d2 loads ride the
sync (HWDGE) queue while d1 loads ride the gpsimd (SWDGE) queue: two
rings keep the SDMA engines busy through each other's completion
latencies (one ring alone sustains only ~340 GB/s on 0.5-1 MB pieces).

Per piece:
  - ScalarE: ln = Ln(d2 + eps), fp8 in -> bf16 out (eps rides the
    activation bias; a 1-wide dummy Ln before the first load pulls the
    ~1.3 us ACT table load into the preamble shadow)
  - the two widest pieces: DVE tensor_mul (bf16 2x mode) + column
    reduce on the otherwise-idle TensorE (ones[128,1].T @ prod into one
    PSUM bank), drained mid-stream by a small DVE tensor_reduce
  - all other pieces: fused DVE scalar_tensor_tensor (1x) whose
    accumulator drops sum(d1*ln) into acc[:, k] directly
Engine budgets: ACT ~16.2 us, DVE ~11.3 us, PE ~7-10 us, DMA ~15 us.
Host sums the [128, 9] fp32 partials of all 8 cores and negates.
"""

import numpy as np
import ml_dtypes

import concourse.bacc as bacc
import concourse.mybir as mybir
import concourse.tile as tile
from concourse.bass_utils import run_bass_kernel_spmd

N = 4096
N_CORES = 8
ROWS_PER_CORE = N // N_CORES  # 512
P = 128
N_TILES = ROWS_PER_CORE // P  # 4 row groups
PIECE_FD = 4096  # max piece width
MM_FD = 512  # one PSUM bank of fp32
EPS = 1e-5

# (row_tile, col_start, width, use_pe) pieces.  Row 0 leads with a
# 512-wide sliver (fast ACT start); row 3 tapers for a short tail.  The
# two widest pieces reduce on TensorE, the rest on DVE's fused STT.
_WIDTHS = {
    0: [512, 3584],
    1: [4096],
    2: [4096],
    3: [2048, 1024, 512, 512],
}
_PIECES = []
for _i in range(N_TILES):
    _c = 0
    for _w in _WIDTHS[_i]:
        _PIECES.append((_i, _c, _w, _w >= 4096))
        _c += _w
    assert _c == N
N_PIECES = len(_PIECES)
N_PE_PIECES = sum(1 for p in _PIECES if p[3])
_LAST_PE_K = max(k for k, p in enumerate(_PIECES) if p[3])
ACC_FD = N_PIECES + 1  # one accum column per STT piece + one for the PSUM drain

_NC_CACHE = {}


def _build_nc():
    nc = bacc.Bacc(
        "TRN2", target_bir_lowering=False, debug=False, num_devices=N_CORES
    )
    d1 = nc.dram_tensor(
        "d1", [ROWS_PER_CORE, N], mybir.dt.bfloat16, kind="ExternalInput"
    )
    d2 = nc.dram_tensor(
        "d2", [ROWS_PER_CORE, N], mybir.dt.float8e4, kind="ExternalInput"
    )
    out = nc.dram_tensor(
        "partial", [P, ACC_FD], mybir.dt.float32, kind="ExternalOutput"
    )
    d1t = d1.rearrange("(n p) m -> n p m", p=P)
    d2t = d2.rearrange("(n p) m -> n p m", p=P)

    with tile.TileContext(nc) as tc:
        with (
            tc.tile_pool(name="p1", bufs=6) as p1,
            tc.tile_pool(name="p2", bufs=6) as p2,
            tc.tile_pool(name="pln", bufs=4) as pln,
            tc.tile_pool(name="pprod", bufs=3) as pprod,
            tc.tile_pool(name="paux", bufs=1) as paux,
            tc.tile_pool(name="psum", bufs=1, space="PSUM") as psum_pool,
        ):
            acc = paux.tile([P, ACC_FD], mybir.dt.float32)
            bias = paux.tile([P, 1], mybir.dt.float32)
            ones = paux.tile([P, 1], mybir.dt.bfloat16)
            warm = paux.tile([P, 1], mybir.dt.bfloat16)
            colsum = psum_pool.tile([1, MM_FD], mybir.dt.float32)
            nc.vector.memset(bias[:], EPS)
            nc.vector.memset(ones[:], 1.0)
            nc.vector.memset(acc[:], 0.0)
            # dummy 1-wide Ln: pulls the ACT table load into the preamble
            # shadow so the first real Ln isn't ~3 us late
            nc.scalar.activation(
                warm[:], ones[:], mybir.ActivationFunctionType.Ln, bias=bias[:, :]
            )
            pe_j = 0  # running PSUM-chunk index across PE pieces
            for k, (i, c0, w, use_pe) in enumerate(_PIECES):
                fs = slice(c0, c0 + w)
                t2 = p2.tile([P, PIECE_FD], mybir.dt.float8e4, tag="t2")
                t1 = p1.tile([P, PIECE_FD], mybir.dt.bfloat16, tag="t1")
                nc.sync.dma_start(t2[:, :w], d2t[i][:, fs])
                nc.gpsimd.dma_start(t1[:, :w], d1t[i][:, fs])
                ln = pln.tile([P, PIECE_FD], mybir.dt.bfloat16, tag="ln")
                nc.scalar.activation(
                    ln[:, :w],
                    t2[:, :w],
                    mybir.ActivationFunctionType.Ln,
                    bias=bias[:, :],
                )
                prod = pprod.tile([P, PIECE_FD], mybir.dt.bfloat16, tag="prod")
                if use_pe:
                    # bf16 2x multiply on DVE; column-sums on the
                    # otherwise-idle TensorE, accumulating in one PSUM bank
                    nc.vector.tensor_mul(prod[:, :w], t1[:, :w], ln[:, :w])
                    for j in range(w // MM_FD):
                        nc.tensor.matmul(
                            colsum[:, :],
                            ones[:, 0:1],
                            prod[:, j * MM_FD : (j + 1) * MM_FD],
                            start=(pe_j == 0),
                            stop=(
                                k == _LAST_PE_K and j == w // MM_FD - 1
                            ),
                        )
                        pe_j += 1
                else:
                    # fused multiply + per-partition reduce (1x, but one
                    # pass): acc[:, k] = sum(d1 * ln)
                    nc.vector.scalar_tensor_tensor(
                        prod[:, :w],
                        t1[:, :w],
                        1.0,
                        ln[:, :w],
                        mybir.AluOpType.mult,
                        mybir.AluOpType.mult,
                        accum_out=acc[:, k : k + 1],
                    )
                if k == _LAST_PE_K + 1:
                    # drain the PE pieces' PSUM bank on DVE mid-stream,
                    # well off the critical tail
                    nc.vector.tensor_reduce(
                        acc[0:1, N_PIECES : N_PIECES + 1],
                        colsum[:, :],
                        axis=mybir.AxisListType.X,
                        op=mybir.AluOpType.add,
                    )
            nc.sync.dma_start(out[:], acc[:])
    nc.compile()
    return nc


def _get_nc():
    if "nc" not in _NC_CACHE:
        _NC_CACHE["nc"] = _build_nc()
    return _NC_CACHE["nc"]


def run_spmd(in_maps, **kwargs):
    """Run the SPMD kernel; returns BassKernelResults (test harness passes
    trace=True kwargs for profiling)."""
    return run_bass_kernel_spmd(
        _get_nc(), in_maps, core_ids=list(range(N_CORES)), **kwargs
    )


def make_in_maps(distribution1, distribution2):
    d1 = np.asarray(distribution1).astype(ml_dtypes.bfloat16)
    d2 = np.asarray(distribution2).astype(ml_dtypes.float8_e4m3)
    in_maps = []
    for c in range(N_CORES):
        sl = slice(c * ROWS_PER_CORE, (c + 1) * ROWS_PER_CORE)
        in_maps.append(
            {
                "d1": np.ascontiguousarray(d1[sl]),
                "d2": np.ascontiguousarray(d2[sl]),
            }
        )
    return in_maps


def reduce_outputs(results):
    total = np.float64(0.0)
    for r in results:
        total += r["partial"].astype(np.float64).sum()
    return np.asarray([-total], dtype=np.float32)


def kernel(distribution1, distribution2):
    in_maps = make_in_maps(distribution1, distribution2)
    res = run_spmd(in_maps)
    return reduce_outputs(res.results)
